# revision 22
# baseline (speedup 1.0000x reference)
"""ACM-GCN (2-layer) distributed Bass kernel for 8 TRN2 NeuronCores.

Strategy:
  - Shard nodes (rows of x / segment ids) across 8 cores: core k owns global
    rows [k*6250, (k+1)*6250), padded to 6272 = 49*128 per core.
  - Layer matmuls (x @ W_*) are local per core (lhsT = pre-transposed x tiles
    shipped from host in bf16).
  - SpMM (adj_low @ xl etc): each core owns the edges whose DESTINATION row
    lives on it. The [xl|xh] activations are cast to fp8e4 and AllGathered
    into a per-core HBM table; per 128-row destination window the kernel
    dma_gathers the source rows (fp8, 256B rows) and segment-sums them with
    one-hot val matmuls accumulated in PSUM. The one-hot lhsT matrices are
    precomputed on the host in fp8 (carrying 64*val to stay in e4m3 normal
    range; the 1/64 is folded into the post-PSUM relu scale) and streamed
    from HBM, so no engine spends time building them.
  - Attention + combine + log_softmax are node-parallel (local), batched at
    layer granularity to minimize DVE instruction count.

All metadata (edge->window assignment, gather indices, one-hot matrices) is
packed host-side in numpy; the Bass graph is static with data-dependent chunk
capacities shared across all 8 cores (max over cores).
"""

import math

import numpy as np
import ml_dtypes

import concourse.bass as bass
import concourse.mybir as mybir
import concourse.tile as tile
from concourse import bacc
from concourse.masks import make_identity

F32 = mybir.dt.float32
BF16 = mybir.dt.bfloat16
F8 = mybir.dt.float8e4
I16 = mybir.dt.int16
BF = ml_dtypes.bfloat16
NF8 = ml_dtypes.float8_e4m3
AF = mybir.ActivationFunctionType
ALU = mybir.AluOpType
AX = mybir.AxisListType

VSCALE = 64.0

DEFAULT_CFG = dict(N=50000, F=512, H=128, C=64, NC=8, GROUP=4, NQ=4)


# --------------------------------------------------------------------------
# Host-side planning / packing
# --------------------------------------------------------------------------

def derive(cfg):
    N, F, H, C, NC = cfg["N"], cfg["F"], cfg["H"], cfg["C"], cfg["NC"]
    assert N % NC == 0
    NSH = N // NC
    PW = (NSH + 127) // 128
    NPAD = PW * 128
    TBL = NC * NPAD
    HALF = TBL // 2
    assert HALF <= 32768, "int16 gather index limit"
    assert F % 128 == 0
    FK = F // 128
    return dict(NSH=NSH, PW=PW, NPAD=NPAD, TBL=TBL, HALF=HALF, FK=FK)


def make_plan(edge_row, edge_col, cfg):
    """Static shape plan shared by all cores: chunk counts per (window, half),
    grouping, chunk bases."""
    d = derive(cfg)
    NC, GROUP = cfg["NC"], cfg["GROUP"]
    NSH, PW, NPAD, HALF = d["NSH"], d["PW"], d["NPAD"], d["HALF"]

    HFL = NPAD // 2
    core = edge_row // NSH
    dr = edge_row - core * NSH
    w = dr // 128
    lr = edge_col % NSH
    half = (lr >= HFL).astype(np.int64)
    key = (core.astype(np.int64) * PW + w) * 2 + half
    counts = np.bincount(key, minlength=NC * PW * 2).reshape(NC, PW, 2)
    gchunks = (counts.max(axis=0) + 127) // 128  # [PW, 2]

    groups = [list(range(g, min(g + GROUP, PW))) for g in range(0, PW, GROUP)]
    chunk_base = {}
    gc = 0
    calls = []  # per group: [(half, total_chunks, base_chunk)]
    for ws in groups:
        gcalls = []
        for hf in (0, 1):
            base = gc
            for wi in ws:
                chunk_base[(wi, hf)] = gc
                gc += int(gchunks[wi, hf])
            gcalls.append((hf, gc - base, base))
        calls.append(gcalls)
    return dict(d=d, gchunks=gchunks, groups=groups, chunk_base=chunk_base,
                calls=calls, GC=gc)


def plan_key(plan, cfg):
    return (tuple(sorted(cfg.items())),
            tuple(map(int, plan["gchunks"].flatten())))


def pack_inputs(inputs, plan, cfg):
    """Build per-core in_maps (numpy) for the bass program."""
    d = plan["d"]
    N, F, H, C, NC = cfg["N"], cfg["F"], cfg["H"], cfg["C"], cfg["NC"]
    NSH, PW, NPAD, HALF, FK = d["NSH"], d["PW"], d["NPAD"], d["HALF"], d["FK"]
    GC = plan["GC"]
    chunk_base = plan["chunk_base"]

    x = np.asarray(inputs["x"], np.float32)
    er = np.asarray(inputs["edge_row"]).astype(np.int64)
    ec = np.asarray(inputs["edge_col"]).astype(np.int64)
    ev = np.asarray(inputs["edge_val"], np.float32)

    w1 = np.concatenate([np.asarray(inputs["weight_low"], np.float32),
                         np.asarray(inputs["weight_high"], np.float32),
                         np.asarray(inputs["weight_mlp"], np.float32)], axis=1)  # [F, 3H]
    w2 = np.concatenate([np.asarray(inputs["weight_low2"], np.float32),
                         np.asarray(inputs["weight_high2"], np.float32),
                         np.asarray(inputs["weight_mlp2"], np.float32)], axis=1)  # [H, 3C]
    # w1 sbuf layout [128, FK, 3H]: [p, kk, n] = w1[kk*128+p, n]
    w1_sb = np.ascontiguousarray(
        w1.reshape(FK, 128, 3 * H).transpose(1, 0, 2)).astype(BF)
    w2_sb = w2.astype(BF)  # [H=128, 3C]

    attl1 = np.stack([np.asarray(inputs["att_vec_low"], np.float32)[:, 0],
                      np.asarray(inputs["att_vec_high"], np.float32)[:, 0],
                      np.asarray(inputs["att_vec_mlp"], np.float32)[:, 0]])  # [3, H]
    attl2 = np.stack([np.asarray(inputs["att_vec_low2"], np.float32)[:, 0],
                      np.asarray(inputs["att_vec_high2"], np.float32)[:, 0],
                      np.asarray(inputs["att_vec_mlp2"], np.float32)[:, 0]])  # [3, C]
    attl1_sb = np.broadcast_to(attl1[None], (128, 3, H)).astype(BF).copy()
    attl2_sb = np.broadcast_to(attl2[None], (128, 3, C)).astype(BF).copy()
    avec1 = (np.asarray(inputs["att_vec"], np.float32) / 3.0).reshape(9)
    avec2 = (np.asarray(inputs["att_vec2"], np.float32) / 3.0).reshape(9)
    avec1_sb = np.broadcast_to(avec1[None], (128, 9)).astype(np.float32).copy()
    avec2_sb = np.broadcast_to(avec2[None], (128, 9)).astype(np.float32).copy()

    # edge metadata
    HFL = NPAD // 2
    core = er // NSH
    dr = er - core * NSH
    w = dr // 128
    rl = (dr % 128).astype(np.int64)
    scl = ec // NSH
    lr = ec % NSH
    half = (lr >= HFL).astype(np.int64)
    idxv = (scl * HFL + lr - half * HFL).astype(np.int64)

    in_maps = []
    for k in range(NC):
        sel = core == k
        kw, khalf, kidx, krl, kval = w[sel], half[sel], idxv[sel], rl[sel], ev[sel]
        # order edges by (w, half)
        order = np.lexsort((khalf, kw))
        kw, khalf, kidx, krl, kval = (a[order] for a in (kw, khalf, kidx, krl, kval))
        seg_key = kw * 2 + khalf
        seg_counts = np.bincount(seg_key, minlength=PW * 2).reshape(PW, 2)
        # global slot for each edge: chunk_base[(w,half)]*128 + position-in-segment
        starts = np.zeros(PW * 2, np.int64)
        np.cumsum(seg_counts.flatten()[:-1], out=starts[1:])
        pos_in_seg = np.arange(len(kw)) - starts[seg_key]
        cb = np.array([[chunk_base[(wi, hf)] for hf in (0, 1)] for wi in range(PW)],
                      np.int64)
        slot = cb[kw, khalf] * 128 + (pos_in_seg // 128) * 128 + pos_in_seg % 128

        idx_flat = np.zeros(GC * 128, np.int16)
        idx_flat[slot] = kidx.astype(np.int16)
        p = slot % 128
        c = slot // 128
        # one-hot matmul lhsT matrices, host-precomputed in fp8:
        # oh[p, c, j] = VSCALE * val for the edge at slot (c*128+p) with local
        # dest row j; zero elsewhere.
        oh = np.zeros((128, GC, 128), NF8)
        oh[p, c, krl] = (kval * VSCALE).astype(NF8)
        # idxs layout for dma_gather: [j%16, j//16] replicated over 8 groups of
        # 16 partitions
        idx16 = np.tile(idx_flat.reshape(-1, 16).T, (8, 1))  # [128, GC*8]

        # pre-transposed x tiles: [PW, 128, FK, 128]:
        # xt[m, p, kk, j] = x[k*NSH + m*128 + j, kk*128 + p]
        xk = np.zeros((NPAD, F), np.float32)
        xk[:NSH] = x[k * NSH:(k + 1) * NSH]
        xt = np.ascontiguousarray(
            xk.reshape(PW, 128, FK, 128).transpose(0, 3, 2, 1)).astype(BF)

        in_maps.append({
            "xt": xt,
            "w1s": w1_sb, "w2s": w2_sb,
            "attl1": attl1_sb, "attl2": attl2_sb,
            "avec1": avec1_sb, "avec2": avec2_sb,
            "idx16": idx16,
            "oh": oh,
        })
    return in_maps


# --------------------------------------------------------------------------
# Bass program
# --------------------------------------------------------------------------

def build_program(plan, cfg, repeat=1):
    d = plan["d"]
    N, F, H, C, NC = cfg["N"], cfg["F"], cfg["H"], cfg["C"], cfg["NC"]
    NSH, PW, NPAD, TBL, HALF, FK = (d["NSH"], d["PW"], d["NPAD"], d["TBL"],
                                    d["HALF"], d["FK"])
    GC = plan["GC"]
    gchunks = plan["gchunks"]
    groups = plan["groups"]
    chunk_base = plan["chunk_base"]
    calls = plan["calls"]
    H2, C2, C3 = 2 * H, 2 * C, 3 * C
    H3 = 3 * H
    TW = 256  # fp8 table row width (elems) for both layers (L2 is padded)
    IVS = 1.0 / VSCALE

    nc = bacc.Bacc(None, target_bir_lowering=False, num_devices=NC,
                   num_swdge_queues=int(cfg.get("NQ", 1)))

    xt_d = nc.declare_dram_parameter("xt", [PW, 128, FK, 128], BF16, isOutput=False)
    w1_d = nc.declare_dram_parameter("w1s", [128, FK, H3], BF16, isOutput=False)
    w2_d = nc.declare_dram_parameter("w2s", [H, C3], BF16, isOutput=False)
    attl1_d = nc.declare_dram_parameter("attl1", [128, 3, H], BF16, isOutput=False)
    attl2_d = nc.declare_dram_parameter("attl2", [128, 3, C], BF16, isOutput=False)
    avec1_d = nc.declare_dram_parameter("avec1", [128, 9], F32, isOutput=False)
    avec2_d = nc.declare_dram_parameter("avec2", [128, 9], F32, isOutput=False)
    idx16_d = nc.declare_dram_parameter("idx16", [128, GC * 8], I16, isOutput=False)
    oh_d = nc.declare_dram_parameter("oh", [128, GC, 128], F8, isOutput=False)
    out_d = nc.declare_dram_parameter("out", [NPAD, C], F32, isOutput=True)

    with tile.TileContext(nc) as tc:
        # ---- DRAM internals
        from contextlib import ExitStack
        es = ExitStack()
        dram_pool = es.enter_context(
            tc.tile_pool(name="dram_pool", bufs=1, space="DRAM"))

        # ---- resident constants / metadata
        consts = es.enter_context(tc.tile_pool(name="consts", bufs=1))
        w1_sb = consts.tile([128, FK, H3], BF16, name="w1_sb")
        nc.sync.dma_start(w1_sb[:], w1_d[:])
        w2_sb = consts.tile([H, C3], BF16, name="w2_sb")
        nc.sync.dma_start(w2_sb[:], w2_d[:])
        attl1_sb = consts.tile([128, 3, H], BF16, name="attl1_sb")
        nc.sync.dma_start(attl1_sb[:], attl1_d[:])
        attl2_sb = consts.tile([128, 3, C], BF16, name="attl2_sb")
        nc.sync.dma_start(attl2_sb[:], attl2_d[:])
        avec1_sb = consts.tile([128, 9], F32, name="avec1_sb")
        nc.sync.dma_start(avec1_sb[:], avec1_d[:])
        avec2_sb = consts.tile([128, 9], F32, name="avec2_sb")
        nc.sync.dma_start(avec2_sb[:], avec2_d[:])
        idx_sb = consts.tile([128, GC * 8], I16, name="idx_sb")
        nc.sync.dma_start(idx_sb[:], idx16_d[:])
        ident = consts.tile([128, 128], BF16, name="ident")
        make_identity(nc, ident[:])

        # ---- resident activations
        res = es.enter_context(tc.tile_pool(name="res", bufs=1))
        xh1_res = res.tile([128, PW, H], BF16, name="xh1_res")
        omlp1_res = res.tile([128, PW, H], BF16, name="omlp1_res")
        xh2_res = res.tile([128, PW, C], BF16, name="xh2_res")
        omlp2_res = res.tile([128, PW, C], BF16, name="omlp2_res")
        olow1_a = res.tile([128, PW, H], BF16, name="olow1_a")
        ohigh1_a = res.tile([128, PW, H], BF16, name="ohigh1_a")
        olow2_a = res.tile([128, PW, C], BF16, name="olow2_a")
        ohigh2_a = res.tile([128, PW, C], BF16, name="ohigh2_a")
        feats1_a = res.tile([128, PW, 3], F32, name="feats1_a")
        feats2_a = res.tile([128, PW, 3], F32, name="feats2_a")
        hc1_a = res.tile([128, PW, H], BF16, name="hc1_a")
        hc2_a = res.tile([128, PW, C], BF16, name="hc2_a")

        # ---- pools
        xt_pool = es.enter_context(tc.tile_pool(name="xt_pool", bufs=3))
        ps1_pool = es.enter_context(tc.tile_pool(name="ps1", bufs=2, space="PSUM"))
        psw_pool = es.enter_context(tc.tile_pool(name="psw", bufs=2, space="PSUM"))
        psT_pool = es.enter_context(tc.tile_pool(name="psT", bufs=2, space="PSUM"))
        ps2_pool = es.enter_context(tc.tile_pool(name="ps2", bufs=2, space="PSUM"))
        g_pool = es.enter_context(tc.tile_pool(name="g_pool", bufs=2))
        oh_pool = es.enter_context(tc.tile_pool(name="oh_pool", bufs=2))
        wtmp_pool = es.enter_context(tc.tile_pool(name="wtmp", bufs=3))
        sm_pool = es.enter_context(tc.tile_pool(name="sm", bufs=2))

        rep_ctr = [0]
        HFL = NPAD // 2
        THL = NC * HFL  # rows per half-table
        mAG1 = (HFL + 127) // 128 - 1  # last window needed for half A
        gsplit = mAG1 // cfg["GROUP"]  # last group of tail-batch 0
        WSPLIT = groups[gsplit][-1] + 1
        NBW = max(WSPLIT, PW - WSPLIT)  # max tail-batch width

        def emit_once():
            rep = rep_ctr[0]
            rep_ctr[0] += 1
            qrr = [0]  # round-robin SWDGE queue counter for gathers
            t1_local = dram_pool.tile([NPAD, TW], F8, name="t1_local",
                                      tag=f"t1l{rep}")
            t1_fA = dram_pool.tile([THL, TW], F8, name="t1_fA",
                                   tag=f"t1fa{rep}", addr_space="Shared")
            t1_fB = dram_pool.tile([THL, TW], F8, name="t1_fB",
                                   tag=f"t1fb{rep}", addr_space="Shared")
            t2_local = dram_pool.tile([NPAD, TW], F8, name="t2_local",
                                      tag=f"t2l{rep}")
            t2_fA = dram_pool.tile([THL, TW], F8, name="t2_fA",
                                   tag=f"t2fa{rep}", addr_space="Shared")
            t2_fB = dram_pool.tile([THL, TW], F8, name="t2_fB",
                                   tag=f"t2fb{rep}", addr_space="Shared")

            def ag(local, full, sel):
                src = local[0:HFL, :] if sel == 0 else local[HFL:NPAD, :]
                nc.gpsimd.collective_compute(
                    "AllGather", ALU.bypass,
                    replica_groups=[list(range(NC))],
                    ins=[src.opt()],
                    outs=[full[:].opt()],
                )

            # ================= Phase A: layer-1 local matmuls =================
            for m in range(PW):
                xt_t = xt_pool.tile([128, FK, 128], BF16, name="xt_t")
                nc.sync.dma_start(xt_t[:], xt_d[m])
                ps = ps1_pool.tile([128, H3], F32, name="ps1_t")
                for kk in range(FK):
                    nc.tensor.matmul(out=ps[:], lhsT=xt_t[:, kk, :],
                                     rhs=w1_sb[:, kk, :],
                                     start=(kk == 0), stop=(kk == FK - 1))
                # [xl|xh] -> fp8 table + local copies
                t1w = wtmp_pool.tile([128, TW], F8, name="t1w", tag="t1w")
                nc.scalar.copy(t1w[:], ps[:, 0:H2])
                nc.sync.dma_start(t1_local[m * 128:(m + 1) * 128, :], t1w[:])
                nc.scalar.activation(xh1_res[:, m, :], ps[:, H:H2], AF.Copy,
                                     scale=VSCALE)
                nc.scalar.activation(omlp1_res[:, m, :], ps[:, H2:H3], AF.Relu)
                if m == mAG1:
                    # half A of the table is complete: AllGather it while the
                    # rest of phase A runs
                    ag(t1_local, t1_fA, 0)
            ag(t1_local, t1_fB, 1)

            # ---- generic window-loop machinery (shared by both layers) ----
            def run_layer(layer, tabA, tabB, ew, xh_res, omlp_res, olow_a,
                          ohigh_a, feats_a, hc_a, attl_sb, avec_sb, sink):
                """layer: 1 or 2; tabA/tabB: DRAM half-tables [THL, TW];
                ew: H or C. xh_res holds VSCALE*xh; olow/ohigh true scale."""
                ew2 = 2 * ew
                nq = int(cfg.get("NQ", 1))

                def tail_batch(b, w0, w1):
                    bw = w1 - w0
                    # ---- attention (batched over windows [w0, w1)) ----
                    sig = sm_pool.tile([128, NBW, 3], F32, name="sig",
                                       tag="sig")
                    sg = sig[:, 0:bw, :]
                    nc.scalar.activation(sg, feats_a[:, w0:w1, :], AF.Sigmoid)
                    zat = sm_pool.tile([128, NBW, 3], F32, name="zat",
                                       tag="zat")
                    za = sm_pool.tile([128, NBW], F32, name="za", tag="za")
                    zb = sm_pool.tile([128, NBW], F32, name="zb", tag="zb")
                    for j in range(3):
                        nc.vector.tensor_scalar(za[:, 0:bw], sig[:, 0:bw, 0],
                                                avec_sb[:, 0 + j:1 + j], None,
                                                ALU.mult)
                        nc.vector.tensor_scalar(zb[:, 0:bw], sig[:, 0:bw, 1],
                                                avec_sb[:, 3 + j:4 + j], None,
                                                ALU.mult)
                        nc.vector.tensor_tensor(out=za[:, 0:bw],
                                                in0=za[:, 0:bw],
                                                in1=zb[:, 0:bw], op=ALU.add)
                        nc.vector.tensor_scalar(zb[:, 0:bw], sig[:, 0:bw, 2],
                                                avec_sb[:, 6 + j:7 + j], None,
                                                ALU.mult)
                        nc.vector.tensor_tensor(out=zat[:, 0:bw, j],
                                                in0=za[:, 0:bw],
                                                in1=zb[:, 0:bw], op=ALU.add)
                    mx = sm_pool.tile([128, NBW], F32, name="mx", tag="mx")
                    nc.vector.tensor_reduce(mx[:, 0:bw], zat[:, 0:bw, :],
                                            axis=AX.X, op=ALU.max)
                    zs = sm_pool.tile([128, NBW, 3], F32, name="zs", tag="zs")
                    nc.vector.tensor_tensor(
                        out=zs[:, 0:bw, :], in0=zat[:, 0:bw, :],
                        in1=mx[:, 0:bw].unsqueeze(2).to_broadcast(
                            [128, bw, 3]),
                        op=ALU.subtract)
                    ez = sm_pool.tile([128, NBW, 3], F32, name="ez", tag="ez")
                    nc.scalar.activation(ez[:, 0:bw, :], zs[:, 0:bw, :],
                                         AF.Exp)
                    ssum = sm_pool.tile([128, NBW], F32, name="ssum",
                                        tag="ssum")
                    nc.vector.tensor_reduce(ssum[:, 0:bw], ez[:, 0:bw, :],
                                            axis=AX.X, op=ALU.add)
                    rs = sm_pool.tile([128, NBW], F32, name="rs", tag="rs")
                    nc.vector.reciprocal(rs[:, 0:bw], ssum[:, 0:bw])
                    nc.vector.tensor_scalar(rs[:, 0:bw], rs[:, 0:bw], 3.0,
                                            None, ALU.mult)
                    att = sm_pool.tile([128, NBW, 3], F32, name="att",
                                       tag="att")
                    nc.vector.tensor_tensor(
                        out=att[:, 0:bw, :], in0=ez[:, 0:bw, :],
                        in1=rs[:, 0:bw].unsqueeze(2).to_broadcast(
                            [128, bw, 3]),
                        op=ALU.mult)
                    # ---- combine (in place; olow/ohigh dead after feats) ----
                    nc.vector.tensor_tensor(
                        out=olow_a[:, w0:w1, :], in0=olow_a[:, w0:w1, :],
                        in1=att[:, 0:bw, 0].unsqueeze(2).to_broadcast(
                            [128, bw, ew]),
                        op=ALU.mult)
                    nc.vector.tensor_tensor(
                        out=ohigh_a[:, w0:w1, :], in0=ohigh_a[:, w0:w1, :],
                        in1=att[:, 0:bw, 1].unsqueeze(2).to_broadcast(
                            [128, bw, ew]),
                        op=ALU.mult)
                    nc.vector.tensor_tensor(out=olow_a[:, w0:w1, :],
                                            in0=olow_a[:, w0:w1, :],
                                            in1=ohigh_a[:, w0:w1, :],
                                            op=ALU.add)
                    nc.vector.tensor_tensor(
                        out=ohigh_a[:, w0:w1, :], in0=omlp_res[:, w0:w1, :],
                        in1=att[:, 0:bw, 2].unsqueeze(2).to_broadcast(
                            [128, bw, ew]),
                        op=ALU.mult)
                    nc.vector.tensor_tensor(out=hc_a[:, w0:w1, :],
                                            in0=olow_a[:, w0:w1, :],
                                            in1=ohigh_a[:, w0:w1, :],
                                            op=ALU.add)
                    sink(b, w0, w1, hc_a)

                for gi, ws in enumerate(groups):
                    nw = len(ws)
                    # gathers for this group (one call per half, rr queues)
                    g_tiles = {}
                    for (hf, nch, base) in calls[gi]:
                        if nch == 0:
                            continue
                        gt = g_pool.tile([128, nch, TW], F8,
                                         name=f"g{layer}_{hf}", tag=f"gt{hf}")
                        src = tabA[:] if hf == 0 else tabB[:]
                        nc.gpsimd.dma_gather(
                            gt[:, :, :], src,
                            idx_sb[:, base * 8:(base + nch) * 8],
                            nch * 128, nch * 128, TW,
                            single_packet=bool(cfg.get("SP1", False)),
                            queue_num=qrr[0] % nq)
                        qrr[0] += 1
                        g_tiles[hf] = (gt, base)
                    # stream this group's one-hot lhsT matrices from HBM
                    gbase = calls[gi][0][2]
                    gtot = sum(nch for (_hf, nch, _b) in calls[gi])
                    oh_g = oh_pool.tile([128, gtot, 128], F8, name="oh_g",
                                        tag="oh_g")
                    nc.sync.dma_start(oh_g[:], oh_d[:, gbase:gbase + gtot, :])
                    for wi, w in enumerate(ws):
                        ps = psw_pool.tile([128, ew2], F32, name="psw_t")
                        spans = []
                        for hf in (0, 1):
                            nch_w = int(gchunks[w, hf])
                            if nch_w == 0 or hf not in g_tiles:
                                continue
                            gt, base = g_tiles[hf]
                            cb = chunk_base[(w, hf)]
                            spans.append((gt, cb - base, cb, nch_w))
                        total = sum(sp[3] for sp in spans)
                        ci = 0
                        for (gt, loff, gcb, nch_w) in spans:
                            for c in range(nch_w):
                                gcc = gcb + c
                                nc.tensor.matmul(out=ps[:],
                                                 lhsT=oh_g[:, gcc - gbase, :],
                                                 rhs=gt[:, loff + c, 0:ew2],
                                                 start=(ci == 0),
                                                 stop=(ci == total - 1))
                                ci += 1
                        if total == 0:
                            nc.vector.memset(ps[:], 0.0)
                        # o_low = relu(S_low) = relu(ps_low) / VSCALE
                        nc.scalar.activation(olow_a[:, w, :], ps[:, 0:ew],
                                             AF.Relu, scale=IVS)
                        # o_high = relu(xh - S_high) = relu(xh64 - ps_high)/VS
                        tmp = wtmp_pool.tile([128, ew], F32, name="ohtmp",
                                             tag="ohtmp")
                        nc.vector.tensor_tensor(out=tmp[:],
                                                in0=xh_res[:, w, :],
                                                in1=ps[:, ew:ew2],
                                                op=ALU.subtract)
                        nc.scalar.activation(ohigh_a[:, w, :], tmp[:],
                                             AF.Relu, scale=IVS)
                    # attention feats (batched per group)
                    pr = wtmp_pool.tile([128, nw, 3, ew], BF16, name="attn_pr",
                                        tag="attn_pr")
                    for j, src_t in enumerate(
                            (olow_a[:, ws[0]:ws[0] + nw, :],
                             ohigh_a[:, ws[0]:ws[0] + nw, :],
                             omlp_res[:, ws[0]:ws[0] + nw, :])):
                        nc.vector.tensor_tensor(
                            out=pr[:, :, j, :], in0=src_t,
                            in1=attl_sb[:, j, :].unsqueeze(1)
                                .to_broadcast([128, nw, ew]),
                            op=ALU.mult)
                    nc.vector.tensor_reduce(feats_a[:, ws[0]:ws[0] + nw, :],
                                            pr[:], axis=AX.X, op=ALU.add)
                    if gi == gsplit:
                        tail_batch(0, 0, WSPLIT)
                tail_batch(1, WSPLIT, PW)

            # ================= Phase C: layer-1 windows =================
            def sink1(b, w0, w1, h_a):
                # transpose h per window, layer-2 local matmul
                for w in range(w0, w1):
                    psT = psT_pool.tile([128, H], BF16, name="psT_t")
                    nc.tensor.transpose(psT[:], h_a[:, w, :], ident[:])
                    hT = wtmp_pool.tile([128, H], BF16, name="hT", tag="hT")
                    nc.scalar.copy(hT[:], psT[:])
                    ps2 = ps2_pool.tile([128, C3], F32, name="ps2_t")
                    nc.tensor.matmul(out=ps2[:], lhsT=hT[:], rhs=w2_sb[:],
                                     start=True, stop=True)
                    t2w = wtmp_pool.tile([128, TW], F8, name="t2w", tag="t2w")
                    nc.scalar.copy(t2w[:, 0:C2], ps2[:, 0:C2])
                    nc.scalar.copy(t2w[:, C2:TW], ps2[:, 0:C2])
                    nc.sync.dma_start(t2_local[w * 128:(w + 1) * 128, :],
                                      t2w[:])
                    nc.scalar.activation(xh2_res[:, w, :], ps2[:, C:C2],
                                         AF.Copy, scale=VSCALE)
                    nc.scalar.activation(omlp2_res[:, w, :], ps2[:, C2:C3],
                                         AF.Relu)
                # half-table AllGathers fire as soon as their rows exist
                if b == 0:
                    ag(t2_local, t2_fA, 0)
                else:
                    ag(t2_local, t2_fB, 1)

            run_layer(1, t1_fA, t1_fB, H, xh1_res, omlp1_res, olow1_a,
                      ohigh1_a, feats1_a, hc1_a, attl1_sb, avec1_sb, sink1)

            # ================= Phase E: layer-2 windows + log_softmax ========
            out_ap = out_d[:].rearrange("(w p) c -> p w c", p=128)

            def sink2(b, w0, w1, h_a):
                mx2 = sm_pool.tile([128, NBW], F32, name="mx2", tag="mx")
                bw = w1 - w0
                nc.vector.tensor_reduce(mx2[:, 0:bw], h_a[:, w0:w1, :],
                                        axis=AX.X, op=ALU.max)
                for ws in groups:
                    if ws[0] < w0 or ws[0] >= w1:
                        continue
                    g0, nw = ws[0], len(ws)
                    dd = sm_pool.tile([128, len(groups[0]), C], F32,
                                      name="dd", tag="dd")
                    ddv = dd[:, 0:nw, :]
                    nc.vector.tensor_tensor(
                        out=ddv, in0=h_a[:, g0:g0 + nw, :],
                        in1=mx2[:, g0 - w0:g0 - w0 + nw].unsqueeze(2)
                            .to_broadcast([128, nw, C]),
                        op=ALU.subtract)
                    exd = sm_pool.tile([128, len(groups[0]), C], F32,
                                       name="exd", tag="exd")
                    exv = exd[:, 0:nw, :]
                    nc.scalar.activation(exv, ddv, AF.Exp)
                    s2 = sm_pool.tile([128, len(groups[0])], F32, name="s2",
                                      tag="s2")
                    nc.vector.tensor_reduce(s2[:, 0:nw], exv, axis=AX.X,
                                            op=ALU.add)
                    ln2 = sm_pool.tile([128, len(groups[0])], F32, name="ln2",
                                       tag="ln2")
                    nc.scalar.activation(ln2[:, 0:nw], s2[:, 0:nw], AF.Ln)
                    nc.vector.tensor_tensor(
                        out=ddv, in0=ddv,
                        in1=ln2[:, 0:nw].unsqueeze(2).to_broadcast(
                            [128, nw, C]),
                        op=ALU.subtract)
                    nc.sync.dma_start(out_ap[:, g0:g0 + nw, :], ddv)

            run_layer(2, t2_fA, t2_fB, C, xh2_res, omlp2_res, olow2_a,
                      ohigh2_a, feats2_a, hc2_a, attl2_sb, avec2_sb, sink2)

        for _rep in range(repeat):
            emit_once()
        es.close()

    nc.compile()
    return nc


# --------------------------------------------------------------------------
# Runner (cached compiled program + jitted PJRT executable)
# --------------------------------------------------------------------------

_CACHE = {}


class _Runner:
    def __init__(self, plan, cfg):
        self.cfg = cfg
        self.plan = plan
        self.nc = build_program(plan, cfg)
        self._fn = None

    def _build_fn(self):
        import jax
        from jax.sharding import Mesh, PartitionSpec
        from jax.experimental.shard_map import shard_map
        from concourse import bass2jax

        nc = self.nc
        NC = self.cfg["NC"]
        bass2jax.install_neuronx_cc_hook()
        partition_name = (nc.partition_id_tensor.name
                          if nc.partition_id_tensor else None)
        in_names, out_names, out_avals, zero_outs = [], [], [], []
        for alloc in nc.m.functions[0].allocations:
            if not isinstance(alloc, mybir.MemoryLocationSet):
                continue
            name = alloc.memorylocations[0].name
            if alloc.kind == "ExternalInput":
                if name != partition_name:
                    in_names.append(name)
            elif alloc.kind == "ExternalOutput":
                shape = tuple(alloc.tensor_shape)
                dtype = mybir.dt.np(alloc.dtype)
                out_avals.append(jax.core.ShapedArray(shape, dtype))
                out_names.append(name)
                zero_outs.append(np.zeros(shape, dtype))
        n_params = len(in_names)
        bind_in_names = list(in_names) + list(out_names)
        if partition_name is not None:
            bind_in_names.append(partition_name)

        def _body(*args):
            operands = list(args)
            if partition_name is not None:
                operands.append(bass2jax.partition_id_tensor())
            outs = bass2jax._bass_exec_p.bind(
                *operands,
                out_avals=tuple(out_avals),
                in_names=tuple(bind_in_names),
                out_names=tuple(out_names),
                lowering_input_output_aliases=(),
                sim_require_finite=True,
                sim_require_nnan=True,
                nc=nc,
            )
            return tuple(outs)

        devices = jax.devices()[:NC]
        mesh = Mesh(np.asarray(devices), ("core",))
        n_outs = len(out_names)
        in_specs = (PartitionSpec("core"),) * (n_params + n_outs)
        out_specs = (PartitionSpec("core"),) * n_outs
        fn = jax.jit(
            shard_map(_body, mesh=mesh, in_specs=in_specs,
                      out_specs=out_specs, check_rep=False),
            keep_unused=True)
        self._fn = fn
        self._in_names = in_names
        self._out_names = out_names
        self._out_avals = out_avals
        self._zero_outs = zero_outs

    def prepare_args(self, in_maps):
        import jax
        NC = self.cfg["NC"]
        per_core = [[np.asarray(m[name]) for name in self._in_names]
                    for m in in_maps]
        concat_in = [np.concatenate([per_core[c][i] for c in range(NC)], axis=0)
                     for i in range(len(self._in_names))]
        concat_zeros = [np.zeros((NC * z.shape[0], *z.shape[1:]), z.dtype)
                        for z in self._zero_outs]
        return [jax.device_put(a) for a in concat_in + concat_zeros]

    def time_ns(self, in_maps, r_hi=6, reps=40):
        """Per-execution device time, measured by differencing wall times of
        this NEFF vs a variant whose body repeats the whole kernel r_hi times
        (fixed RPC + input-staging costs cancel in the difference)."""
        import time
        import jax

        if self._fn is None:
            self._build_fn()
        if not hasattr(self, "_fn_hi") or self._fn_hi is None:
            rh = _Runner.__new__(_Runner)
            rh.cfg = self.cfg
            rh.plan = self.plan
            rh.nc = build_program(self.plan, self.cfg, repeat=r_hi)
            rh._fn = None
            rh._build_fn()
            self._fn_hi = rh._fn
            self._rh = rh
            self._r_hi = r_hi
        a1 = self.prepare_args(in_maps)
        ah = self._rh.prepare_args(in_maps)
        jax.block_until_ready(self._fn(*a1))
        jax.block_until_ready(self._fn_hi(*ah))
        t1s, ths = [], []
        for _ in range(reps):
            t0 = time.perf_counter()
            jax.block_until_ready(self._fn(*a1))
            t1s.append(time.perf_counter() - t0)
            t0 = time.perf_counter()
            jax.block_until_ready(self._fn_hi(*ah))
            ths.append(time.perf_counter() - t0)
        t1s.sort()
        ths.sort()
        i = max(1, reps // 10)
        return (ths[i] - t1s[i]) / (self._r_hi - 1) * 1e9

    def run(self, in_maps):
        import jax
        if self._fn is None:
            self._build_fn()
        args = self.prepare_args(in_maps)
        outs = self._fn(*args)
        jax.block_until_ready(outs)
        NC = self.cfg["NC"]
        res = []
        for c in range(NC):
            m = {}
            for i, name in enumerate(self._out_names):
                m[name] = np.asarray(outs[i]).reshape(
                    NC, *self._out_avals[i].shape)[c]
            res.append(m)
        return res


def get_runner(inputs, cfg=None):
    cfg = dict(DEFAULT_CFG if cfg is None else cfg)
    plan = make_plan(np.asarray(inputs["edge_row"]).astype(np.int64),
                     np.asarray(inputs["edge_col"]).astype(np.int64), cfg)
    key = plan_key(plan, cfg)
    if key not in _CACHE:
        _CACHE[key] = _Runner(plan, cfg)
    return _CACHE[key], plan


def kernel(**inputs) -> np.ndarray:
    cfg = dict(DEFAULT_CFG)
    runner, plan = get_runner(inputs, cfg)
    in_maps = pack_inputs(inputs, runner.plan, cfg)
    res = runner.run(in_maps)
    NSH = runner.plan["d"]["NSH"]
    out = np.concatenate([res[k]["out"][:NSH] for k in range(cfg["NC"])],
                         axis=0)
    return out[:cfg["N"]].astype(np.float32)
